# revision 21
# baseline (speedup 1.0000x reference)
"""Trainium2 Bass kernel for ContrastiveMaskedPatchSimilarity loss.

Computes: per-position cosine similarity along the channel axis of two
[32, 256, 64, 64] f32 tensors, then a masked mean -> scalar.

Strategy (pure data parallel over 8 NeuronCores, batch-sharded 4 each):
  - Inputs are cast to fp16 on the host before upload: HBM traffic halves
    (the kernel is DMA-bound) and fp16's 11-bit mantissa keeps the final
    rel-err ~1.6e-4 (better than bf16 products from f32, which give 6.7e-4).
  - Layout on chip: [channel-chunk (128) = partitions, spatial (4096) = free].
    DMA tiles are contiguous 8KB-per-partition lines; u streams on the sync
    queue (HWDGE), m on the gpsimd queue (SWDGE); the two queues together
    run at the ~320 GB/s per-core HBM cap.
  - Elementwise products (u*m, u*u, m*m) -> fp16: DVE does num+mm in 2x
    16-bit mode (~2.3us/tile), ACT does uu (~3.7us/tile, dtype-independent).
    Per chunk DVE emits mm BEFORE num so the PE can start the mm columns
    (its first stat group) as early as possible.
  - Channel reduction via TensorE: product slice [128ch x 128pos] is the
    stationary operand, rhs = ones[128,1] fp16 -> out[128pos, 1] in PSUM.
    The two channel chunks accumulate into the same PSUM column
    (start/stop flags), so the epilogue needs no cross-chunk adds.
  - The last chunk is split into two position halves so the end-of-kernel
    serial chain (products -> matmuls -> epilogue) is half as deep.
  - Epilogue per batch (delayed one batch so ACT never stalls on PE):
    two parallel chains -- [copy num; *mask] and [copy uu; *mm; 1/x; sqrt]
    -- joined by a final multiply + free-axis reduce into acc[:, b].
  - Host: sum partials over cores/partitions, divide by host-computed
    mask count (exact).
"""

import sys
from contextlib import ExitStack

import numpy as np

sys.path.insert(0, "/opt/trn_rl_repo")

import concourse.bass as bass  # noqa: E402
import concourse.tile as tile  # noqa: E402
from concourse import bacc, mybir  # noqa: E402
from concourse.bass_utils import run_bass_kernel_spmd  # noqa: E402

B, C, H, W = 32, 256, 64, 64
NCORES = 8
BL = B // NCORES  # batches per core: 4
HWX = H * W  # 4096
ROWS = BL * C  # 1024
NPB = HWX // 128  # position blocks per batch: 32
NCHUNK = C // 128  # channel chunks: 2

F32 = mybir.dt.float32
F16 = mybir.dt.float16

_CACHED_NC = None


def build_nc():
    nc = bacc.Bacc(
        "TRN2", target_bir_lowering=False, debug=False, num_devices=NCORES
    )
    u_d = nc.dram_tensor("u", [ROWS, HWX], F16, kind="ExternalInput")
    m_d = nc.dram_tensor("m", [ROWS, HWX], F16, kind="ExternalInput")
    # mask, pre-transposed on host to [p_in (128), b*NPB + pb (128)] f32
    mk_d = nc.dram_tensor("maskf", [128, BL * NPB], F32, kind="ExternalInput")
    ones_d = nc.dram_tensor("ones", [128, 1], F16, kind="ExternalInput")
    # out[:, b] = per-batch sum(sim*mask) partials (per partition)
    out_d = nc.dram_tensor("out", [128, BL], F32, kind="ExternalOutput")

    with tile.TileContext(nc) as tc, ExitStack() as ctx:
        const_pool = ctx.enter_context(tc.tile_pool(name="const", bufs=1))
        in_pool = ctx.enter_context(tc.tile_pool(name="inp", bufs=6))
        prod_pool = ctx.enter_context(tc.tile_pool(name="prod", bufs=2))
        ep_pool = ctx.enter_context(tc.tile_pool(name="ep", bufs=2))
        acc_pool = ctx.enter_context(tc.tile_pool(name="acc", bufs=1))
        psum_pool = ctx.enter_context(
            tc.tile_pool(name="psum", bufs=3, space="PSUM")
        )

        ones_t = const_pool.tile([128, 1], F16)
        nc.sync.dma_start(ones_t[:], ones_d[:, :])
        maskf_t = const_pool.tile([128, BL * NPB], F32)
        nc.sync.dma_start(maskf_t[:], mk_d[:, :])
        acc_t = acc_pool.tile([128, BL], F32)

        def epilogue(b, P):
            # PSUM cols of P: num [0:NPB], uu [NPB:2NPB], mm [2NPB:3NPB]
            # Two parallel chains; DVE reads at most one PSUM operand per op.
            nn = ep_pool.tile([128, NPB], F32, tag="nn")
            nc.scalar.copy(nn[:], P[:, 0:NPB])
            us = ep_pool.tile([128, NPB], F32, tag="us")
            nc.scalar.copy(us[:], P[:, NPB : 2 * NPB])
            sm0 = ep_pool.tile([128, NPB], F32, tag="sm0")
            nc.vector.tensor_mul(
                sm0[:], nn[:], maskf_t[:, b * NPB : (b + 1) * NPB]
            )
            d2 = ep_pool.tile([128, NPB], F32, tag="d2")
            nc.vector.tensor_mul(d2[:], us[:], P[:, 2 * NPB : 3 * NPB])
            r = ep_pool.tile([128, NPB], F32, tag="r")
            nc.vector.reciprocal(r[:], d2[:])
            rs = ep_pool.tile([128, NPB], F32, tag="rs")
            nc.scalar.sqrt(rs[:], r[:])
            sm = ep_pool.tile([128, NPB], F32, tag="sm")
            nc.vector.tensor_mul(sm[:], sm0[:], rs[:])
            nc.vector.tensor_reduce(
                acc_t[:, b : b + 1],
                sm[:],
                axis=mybir.AxisListType.X,
                op=mybir.AluOpType.add,
            )

        def load_and_products(row0, cols, tagsuf="", nbufs=None):
            """DMA one [128, len(cols)] chunk of u and m, emit mm/num on DVE
            and uu on ACT. Returns (num, uu, mm) product tiles. Sub-size
            chunks pass tagsuf so they get their own pool slots -- sharing a
            tag with full tiles aliases addresses PE still has to read and
            serializes the tail."""
            n = cols.stop - cols.start
            qa, qb = nc.sync, nc.gpsimd
            u_t = in_pool.tile([128, n], F16, tag="u" + tagsuf, bufs=nbufs)
            qa.dma_start(u_t[:], u_d[row0 : row0 + 128, cols])
            m_t = in_pool.tile([128, n], F16, tag="m" + tagsuf, bufs=nbufs)
            qb.dma_start(m_t[:], m_d[row0 : row0 + 128, cols])
            mm_t = prod_pool.tile([128, n], F16, tag="mm" + tagsuf,
                                  bufs=nbufs)
            nc.vector.tensor_mul(mm_t[:], m_t[:], m_t[:])
            num_t = prod_pool.tile([128, n], F16, tag="num" + tagsuf,
                                   bufs=nbufs)
            nc.vector.tensor_mul(num_t[:], u_t[:], m_t[:])
            uu_t = prod_pool.tile([128, n], F16, tag="uu" + tagsuf,
                                  bufs=nbufs)
            nc.scalar.square(uu_t[:], u_t[:])
            return (num_t, uu_t, mm_t)

        def matmul_pairs(P, prods0, prods1, pbs, pb_off1):
            """Per PSUM column: ch0-stationary matmul (start) then ch1
            (stop), emitting stats in DVE-completion order mm, uu, num."""
            for s in (2, 1, 0):
                for pb in pbs:
                    col = s * NPB + pb
                    c0 = slice(pb * 128, (pb + 1) * 128)
                    c1 = slice(
                        (pb - pb_off1) * 128, (pb - pb_off1 + 1) * 128
                    )
                    nc.tensor.matmul(
                        P[:, col : col + 1],
                        prods0[s][:, c0],
                        ones_t[:, :],
                        start=True,
                        stop=False,
                    )
                    nc.tensor.matmul(
                        P[:, col : col + 1],
                        prods1[s][:, c1],
                        ones_t[:, :],
                        start=False,
                        stop=True,
                    )

        pend = []  # (b, P) awaiting epilogue
        for b in range(BL):
            P = psum_pool.tile([128, 3 * NPB], F32)
            row0 = b * C
            prods0 = load_and_products(row0, slice(0, HWX))
            if pend:
                epilogue(*pend.pop())
            last = b == BL - 1
            if not last:
                prods1 = load_and_products(row0 + 128, slice(0, HWX))
                matmul_pairs(P, prods0, prods1, range(NPB), 0)
            else:
                # final chunk split 3/4 + 1/4: the exposed products->
                # matmuls->epilogue chain after the last byte lands only
                # spans the last quarter (two sub-chunks, so the "h" tag's
                # two bufs never recycle -- no slot-aliasing stall)
                SPLIT = (3 * HWX) // 4
                SPB = (3 * NPB) // 4
                prods1a = load_and_products(
                    row0 + 128, slice(0, SPLIT), "h", nbufs=2
                )
                matmul_pairs(P, prods0, prods1a, range(SPB), 0)
                prods1b = load_and_products(
                    row0 + 128, slice(SPLIT, HWX), "h", nbufs=2
                )
                matmul_pairs(P, prods0, prods1b, range(SPB, NPB), SPB)
            pend.append((b, P))

        epilogue(*pend.pop())
        nc.sync.dma_start(out_d[:, :], acc_t[:])

    nc.compile()
    return nc


def get_nc():
    global _CACHED_NC
    if _CACHED_NC is None:
        _CACHED_NC = build_nc()
    return _CACHED_NC


def make_in_maps(unmasked, masked, latent_mask):
    ones = np.ones((128, 1), dtype=np.float16)
    u16 = unmasked.astype(np.float16)
    m16 = masked.astype(np.float16)
    in_maps = []
    for i in range(NCORES):
        sl = slice(i * BL, (i + 1) * BL)
        u = np.ascontiguousarray(u16[sl]).reshape(ROWS, HWX)
        m = np.ascontiguousarray(m16[sl]).reshape(ROWS, HWX)
        mk = latent_mask[sl].reshape(128, 128).T.astype(np.float32)
        in_maps.append(
            {
                "u": u,
                "m": m,
                "maskf": np.ascontiguousarray(mk),
                "ones": ones,
            }
        )
    return in_maps


def _finalize(results, latent_mask):
    num = 0.0
    for res in results:
        num += np.asarray(res["out"], dtype=np.float64).sum()
    den = float((latent_mask != 0).sum())
    return np.float32(num / den)


def kernel(unmasked_latent_tensors, masked_latent_tensors, latent_mask, **kw):
    nc = get_nc()
    lm = np.asarray(latent_mask)
    in_maps = make_in_maps(
        np.asarray(unmasked_latent_tensors, dtype=np.float32),
        np.asarray(masked_latent_tensors, dtype=np.float32),
        lm,
    )
    res = run_bass_kernel_spmd(nc, in_maps, list(range(NCORES)))
    return _finalize(res.results, lm)


def kernel_traced(unmasked_latent_tensors, masked_latent_tensors, latent_mask):
    """Like kernel() but with NTFF tracing; returns (value, BassKernelResults)."""
    nc = get_nc()
    lm = np.asarray(latent_mask)
    in_maps = make_in_maps(
        np.asarray(unmasked_latent_tensors, dtype=np.float32),
        np.asarray(masked_latent_tensors, dtype=np.float32),
        lm,
    )
    res = run_bass_kernel_spmd(nc, in_maps, list(range(NCORES)), trace=True)
    return _finalize(res.results, lm), res
